# revision 14
# baseline (speedup 1.0000x reference)
"""Balanced BCE loss on 8 Trainium2 NeuronCores.

loss = -sum_i [ beta_i * sum_j(t_ij * ln(p_ij))
                + (1-beta_i) * sum_j((1-t_ij) * ln(1-p_ij)) ]
beta_i = 1 - mean_j(t_ij)

Per-core row statistics (8 batch rows per core):
  S=sum(t)  A=sum(t*lnp)  C=sum(t*ln1mp)  B=sum(ln1mp)
host combines: loss = -sum_rows[ beta*A + (1-beta)*(B-C) ], beta = 1-S/N

Engine assignment per row tile [128, F]:
  - ACT: lnp = Ln(p) bf16; ln1mp = Ln(1-p) bf16 with accum_out -> B
  - DVE: cast t->bf16 (2x); m1 = t*lnp, m2 = t*ln1mp (bf16 TT, 2x)
  - PE: one-hot-weight chunk matmuls accumulate S, A and C into
    per-row PSUM partitions psX[8, 512]; banks rotate S,A,C per chunk.
  - End: S second level on ACT (copy+accum), A/C on DVE, B-fold on PE.

DMA schedule: the p tensor streams on the Sync HWDGE queue while the t
tensor streams concurrently on the GpSimd SWDGE queue - two independent
rings, so both halves of the 16MB finish together instead of t7
trailing a serialized 16-transfer FIFO.  8 DMAs per queue all admit
into the completion-semaphore window immediately.  Row 7 is processed
in column quarters (its quarters accumulate into the same one-hot PSUM
row), so the post-stream tail is one quarter's worth of work.
"""

from contextlib import ExitStack

import numpy as np

import concourse.bass as bass
import concourse.mybir as mybir
import concourse.tile as tile
from concourse import bacc
from concourse.bass_utils import run_bass_kernel_spmd

B, N = 64, 262144
NCORES = 8
ROWS = B // NCORES  # rows per core
P = 128  # SBUF partitions
F = N // P  # 2048 elements per partition per row
QF = F // 4
CH = 512  # matmul moving-dim chunk (max for f32 PSUM)
NB = ROWS + 3  # B columns: rows 0-6 plus 4 row-7 quarters

AF = mybir.ActivationFunctionType
ALU = mybir.AluOpType
AX = mybir.AxisListType
f32 = mybir.dt.float32
bf16 = mybir.dt.bfloat16

# test.py can flip this to capture an NTFF profile of the run
TRACE = False
LAST = None  # BassKernelResults of the most recent kernel() call


def _emit(tc, out_ac, out_sb, inp_ap, tgt_ap):
    nc = tc.nc
    rows = ROWS

    with ExitStack() as ctx:
        io_pool = ctx.enter_context(tc.tile_pool(name="io", bufs=rows))
        bf_pool = ctx.enter_context(tc.tile_pool(name="bf", bufs=2))
        bfq_pool = ctx.enter_context(tc.tile_pool(name="bfq", bufs=2))
        tb_pool = ctx.enter_context(tc.tile_pool(name="tb", bufs=3))
        psum_pool = ctx.enter_context(tc.tile_pool(name="ps", bufs=1, space="PSUM"))
        singles = ctx.enter_context(tc.tile_pool(name="const", bufs=1))

        oh = singles.tile([P, rows * rows], bf16, tag="oh")
        nc.vector.memset(oh[:], 0.0)
        for v in range(rows):
            nc.vector.memset(oh[:, v * rows + v : v * rows + v + 1], 1.0)
        ones_f = singles.tile([P, 1], f32, tag="ones_f")
        nc.vector.memset(ones_f[:], 1.0)
        accB = singles.tile([P, NB], f32, tag="accB")
        ac_sb = singles.tile([rows, 3], f32, tag="ac_sb")  # cols: S, A, C
        sb_sb = singles.tile([1, NB], f32, tag="sb_sb")  # B row
        junk = singles.tile([rows, CH], f32, tag="junk")

        inp3 = inp_ap.rearrange("r (p f) -> p r f", p=P)
        tgt3 = tgt_ap.rearrange("r (p f) -> p r f", p=P)

        psS = psum_pool.tile([rows, CH], f32, tag="psS", name="psS")
        psA = psum_pool.tile([rows, CH], f32, tag="psA", name="psA")
        psC = psum_pool.tile([rows, CH], f32, tag="psC", name="psC")
        psB = psum_pool.tile([1, NB], f32, tag="psB", name="psB")

        # p rows on the Sync HWDGE ring, t rows concurrently on the GpSimd
        # SWDGE ring; row 7 as quarters on both
        ptiles, ttiles = [], []
        for r in range(rows):
            pp = io_pool.tile([P, F], f32, tag="p", name=f"pp_{r}")
            tt = io_pool.tile([P, F], f32, tag="t", name=f"tt_{r}")
            ptiles.append(pp)
            ttiles.append(tt)
        for r in range(rows - 1):
            nc.sync.dma_start(ptiles[r][:], inp3[:, r, :])
        for q in range(4):
            sl = slice(q * QF, (q + 1) * QF)
            nc.sync.dma_start(ptiles[rows - 1][:, sl], inp3[:, rows - 1, sl])
        for r in range(rows - 1):
            nc.gpsimd.dma_start(ttiles[r][:], tgt3[:, r, :])
        for q in range(4):
            sl = slice(q * QF, (q + 1) * QF)
            nc.gpsimd.dma_start(ttiles[rows - 1][:, sl], tgt3[:, rows - 1, sl])

        nch = F // CH

        def do_unit(p_t, t_t, ohv, w, bcol, pool, first, last):
            """One compute unit of width w: 2 Lns, cast, 2 muls, 3*w/CH MMs."""
            logp = pool.tile([P, w], bf16, tag=f"logp{w}", name="logp")
            nc.scalar.activation(logp[:], p_t, AF.Ln)
            l1mp = pool.tile([P, w], bf16, tag=f"l1mp{w}", name="l1mp")
            nc.scalar.activation(
                l1mp[:], p_t, AF.Ln, scale=-1.0, bias=1.0,
                accum_out=accB[:, bcol : bcol + 1],
            )
            tb = tb_pool.tile([P, w], bf16, tag=f"tb{w}", name="tb")
            nc.vector.tensor_copy(tb[:], t_t)
            m1 = pool.tile([P, w], bf16, tag=f"m{w}", name="m1")
            nc.vector.tensor_mul(m1[:], tb[:], logp[:])
            m2 = pool.tile([P, w], bf16, tag=f"m{w}", name="m2")
            nc.vector.tensor_mul(m2[:], tb[:], l1mp[:])
            for c in range(w // CH):
                for k, (ps, src) in enumerate(((psS, tb), (psA, m1), (psC, m2))):
                    nc.tensor.matmul(
                        ps[:, :],
                        ohv,
                        src[:, c * CH : (c + 1) * CH],
                        start=(first and c == 0),
                        stop=(last and c == w // CH - 1),
                        skip_group_check=True,
                    )

        for r in range(rows - 1):
            ohv = oh[:, r * rows : (r + 1) * rows]
            do_unit(ptiles[r][:], ttiles[r][:], ohv, F, min(r, NB - 1),
                    bf_pool, first=(r == 0), last=False)
        r = rows - 1
        ohv = oh[:, r * rows : (r + 1) * rows]
        for q in range(4):
            sl = slice(q * QF, (q + 1) * QF)
            do_unit(ptiles[r][:, sl], ttiles[r][:, sl], ohv, QF, 7 + q,
                    bfq_pool, first=False, last=(q == 3))

        # second level: S on ACT (copy+accum), A/C on DVE, B-fold on PE
        nc.scalar.activation(
            junk[:, :], psS[:, :], AF.Copy, accum_out=ac_sb[:, 0:1]
        )
        nc.vector.tensor_reduce(ac_sb[:, 1:2], psA[:, :], axis=AX.X, op=ALU.add)
        nc.vector.tensor_reduce(ac_sb[:, 2:3], psC[:, :], axis=AX.X, op=ALU.add)
        nc.tensor.matmul(psB[0:1, :], ones_f[:], accB[:, :], start=True, stop=True)
        nc.vector.tensor_copy(sb_sb[0:1, :], psB[0:1, :])
        nc.sync.dma_start(out_ac, ac_sb[:])
        nc.sync.dma_start(out_sb, sb_sb[:])


_PROG_CACHE = {}


def _build_program():
    key = (ROWS, N)
    if key not in _PROG_CACHE:
        nc = bacc.Bacc("TRN2", target_bir_lowering=False, debug=False)
        inp = nc.dram_tensor("input", [ROWS, N], f32, kind="ExternalInput").ap()
        tgt = nc.dram_tensor("target", [ROWS, N], f32, kind="ExternalInput").ap()
        oac = nc.dram_tensor("partialsAC", [ROWS, 3], f32, kind="ExternalOutput").ap()
        osb = nc.dram_tensor("partialsSB", [1, NB], f32, kind="ExternalOutput").ap()
        with tile.TileContext(nc) as tc:
            _emit(tc, oac, osb, inp, tgt)
        nc.finalize()
        _PROG_CACHE[key] = nc
    return _PROG_CACHE[key]


def kernel(input, target):
    global LAST
    input = np.ascontiguousarray(np.asarray(input))
    target = np.ascontiguousarray(np.asarray(target))
    assert input.shape == (B, N) and target.shape == (B, N)

    nc = _build_program()
    in_maps = [
        {
            "input": input[c * ROWS : (c + 1) * ROWS],
            "target": target[c * ROWS : (c + 1) * ROWS],
        }
        for c in range(NCORES)
    ]
    res = run_bass_kernel_spmd(nc, in_maps, core_ids=list(range(NCORES)), trace=TRACE)
    LAST = res

    total = np.float64(0.0)
    for c in range(NCORES):
        ac = res.results[c]["partialsAC"].astype(np.float64)  # [8, 3] = S, A, C
        braw = res.results[c]["partialsSB"].astype(np.float64).reshape(NB)
        Bv = np.concatenate([braw[:7], [braw[7:11].sum()]])
        S, A, C = ac[:, 0], ac[:, 1], ac[:, 2]
        beta = 1.0 - S / N
        total += np.sum(beta * A + (1.0 - beta) * (Bv - C))
    return np.float32(-total)


# revision 17
# speedup vs baseline: 1.2864x; 1.2864x over previous
"""Balanced BCE loss on 8 Trainium2 NeuronCores.

loss = -sum_i [ beta_i * sum_j(t_ij * ln(p_ij))
                + (1-beta_i) * sum_j((1-t_ij) * ln(1-p_ij)) ]
beta_i = 1 - mean_j(t_ij)

Per-core row statistics (8 batch rows per core):
  S=sum(t)  A=sum(t*lnp)  C=sum(t*ln1mp)  B=sum(ln1mp)
host combines: loss = -sum_rows[ beta*A + (1-beta)*(B-C) ], beta = 1-S/N

Engine assignment per row tile [128, 2048]:
  - ACT: lnp = Ln(p) bf16; ln1mp = Ln(1-p) bf16 with accum_out -> B per partition
  - DVE: cast t->bf16 (2x mode); m1 = t*lnp, m2 = t*ln1mp (bf16 TT, 2x mode)
  - PE: ones-matmul chunk reductions of m1/m2 and tb (bf16), plus one
        tiny matmul for the cross-partition reduce of B
"""

from contextlib import ExitStack

import numpy as np

import concourse.bass as bass
import concourse.mybir as mybir
import concourse.tile as tile
from concourse import bacc
from concourse.bass_utils import run_bass_kernel_spmd

B, N = 64, 262144
NCORES = 8
ROWS = B // NCORES  # rows per core
P = 128  # SBUF partitions

AF = mybir.ActivationFunctionType
ALU = mybir.AluOpType
f32 = mybir.dt.float32
bf16 = mybir.dt.bfloat16

# test.py can flip this to capture an NTFF profile of the run
TRACE = False
LAST = None  # BassKernelResults of the most recent kernel() call


def _emit(tc, out_ap, inp_ap, tgt_ap, rows, n):
    """Emit the per-core program. out_ap: [1, 4*rows] f32 = [S.., B.., A.., C..]."""
    nc = tc.nc
    F = n // P
    CH = 128  # matmul moving-dim chunk; per-row PSUM stripe is CH wide
    nch = F // CH
    assert nch * CH == F

    with ExitStack() as ctx:
        io_pool = ctx.enter_context(tc.tile_pool(name="io", bufs=rows))
        bf_pool = ctx.enter_context(tc.tile_pool(name="bf", bufs=3))
        psum_pool = ctx.enter_context(tc.tile_pool(name="ps", bufs=1, space="PSUM"))
        singles = ctx.enter_context(tc.tile_pool(name="const", bufs=1))

        ones_bf = singles.tile([P, 1], bf16, tag="ones_bf")
        nc.vector.memset(ones_bf[:], 1.0)
        ones_f = singles.tile([P, 1], f32, tag="ones_f")
        nc.vector.memset(ones_f[:], 1.0)
        accB = singles.tile([P, rows], f32, tag="accB")
        stats = singles.tile([1, 4 * rows], f32, tag="stats")
        junk = singles.tile([1, 128], f32, tag="junk")

        inp3 = inp_ap.rearrange("r (p f) -> p r f", p=P)
        tgt3 = tgt_ap.rearrange("r (p f) -> p r f", p=P)

        # psA/psC/psS: 2 PSUM banks each; psB gets its own bank
        psA = psum_pool.tile([1, rows * CH], f32, tag="psA", name="psA")
        psC = psum_pool.tile([1, rows * CH], f32, tag="psC", name="psC")
        psS = psum_pool.tile([1, rows * CH], f32, tag="psS", name="psS")
        psB = psum_pool.tile([1, rows], f32, tag="psB", name="psB")

        # per-row 1MB loads on the otherwise-idle SP engine, all triggers
        # emitted upfront (first io_bufs rows stream immediately; later
        # triggers wait inline on slot recycling, which only stalls SP).
        # The last t row is split in half so its consumer chain starts
        # ~1us earlier.
        ptiles, ttiles = [], []
        for r in range(rows):
            pp = io_pool.tile([P, F], f32, tag="p", name=f"pp_{r}")
            nc.sync.dma_start(pp[:], inp3[:, r, :])
            ptiles.append(pp)
            tt = io_pool.tile([P, F], f32, tag="t", name=f"tt_{r}")
            nc.sync.dma_start(tt[:], tgt3[:, r, :])
            ttiles.append(tt)

        for r in range(rows):
            p_t = ptiles[r][:]
            t_t = ttiles[r][:]

            logp = bf_pool.tile([P, F], bf16, tag="logp")
            nc.scalar.activation(logp[:], p_t, AF.Ln)
            l1mp = bf_pool.tile([P, F], bf16, tag="l1mp")
            nc.scalar.activation(
                l1mp[:], p_t, AF.Ln, scale=-1.0, bias=1.0,
                accum_out=accB[:, r : r + 1],
            )

            tb = bf_pool.tile([P, F], bf16, tag="tb")
            nc.vector.tensor_copy(tb[:], t_t)
            m1 = bf_pool.tile([P, F], bf16, tag="m1")
            nc.vector.tensor_mul(m1[:], tb[:], logp[:])
            m2 = bf_pool.tile([P, F], bf16, tag="m2")
            nc.vector.tensor_mul(m2[:], tb[:], l1mp[:])

            for ps, src in ((psS, tb), (psA, m1), (psC, m2)):
                for c in range(nch):
                    nc.tensor.matmul(
                        ps[0:1, r * CH : (r + 1) * CH],
                        ones_bf[:],
                        src[:, c * CH : (c + 1) * CH],
                        start=(c == 0),
                        stop=(c == nch - 1),
                    )

            # per-row second-level reduces: S on ACT (Copy+accum), A/C on
            # DVE (keeps ~10us of Copy+accum-read work off the ACT queue,
            # which otherwise binds the whole pipeline)
            nc.scalar.activation(
                junk[0:1, :CH],
                psS[0:1, r * CH : (r + 1) * CH],
                AF.Copy,
                accum_out=stats[0:1, r : r + 1],
            )
            for ps, col in ((psA, 2 * rows + r), (psC, 3 * rows + r)):
                nc.vector.tensor_reduce(
                    stats[0:1, col : col + 1],
                    ps[0:1, r * CH : (r + 1) * CH],
                    axis=mybir.AxisListType.X,
                    op=ALU.add,
                )

        # cross-partition reduce of B accumulators on PE
        nc.tensor.matmul(psB[0:1, :], ones_f[:], accB[:, :])
        nc.vector.tensor_copy(stats[0:1, rows : 2 * rows], psB[0:1, :])
        nc.sync.dma_start(out_ap, stats[:])


_PROG_CACHE = {}


def _build_program(rows=ROWS, n=N):
    key = (rows, n)
    if key not in _PROG_CACHE:
        nc = bacc.Bacc("TRN2", target_bir_lowering=False, debug=False)
        inp = nc.dram_tensor("input", [rows, n], f32, kind="ExternalInput").ap()
        tgt = nc.dram_tensor("target", [rows, n], f32, kind="ExternalInput").ap()
        out = nc.dram_tensor("partials", [1, 4 * rows], f32, kind="ExternalOutput").ap()
        with tile.TileContext(nc) as tc:
            _emit(tc, out, inp, tgt, rows, n)
        nc.finalize()
        _PROG_CACHE[key] = nc
    return _PROG_CACHE[key]


def kernel(input, target):
    global LAST
    input = np.ascontiguousarray(np.asarray(input))
    target = np.ascontiguousarray(np.asarray(target))
    assert input.shape == (B, N) and target.shape == (B, N)

    nc = _build_program()
    in_maps = [
        {
            "input": input[c * ROWS : (c + 1) * ROWS],
            "target": target[c * ROWS : (c + 1) * ROWS],
        }
        for c in range(NCORES)
    ]
    res = run_bass_kernel_spmd(nc, in_maps, core_ids=list(range(NCORES)), trace=TRACE)
    LAST = res

    total = np.float64(0.0)
    for c in range(NCORES):
        part = res.results[c]["partials"].astype(np.float64).reshape(4, ROWS)
        S, Bv, A, C = part[0], part[1], part[2], part[3]
        beta = 1.0 - S / N
        total += np.sum(beta * A + (1.0 - beta) * (Bv - C))
    return np.float32(-total)

